# revision 23
# baseline (speedup 1.0000x reference)
"""Trainium2 Bass kernel for nn_Encoder_WordLstm (bi-LSTM over char/bichar embeddings).

Sharding: data-parallel over batch. Each of the 8 cores handles 8 sentences and
runs BOTH LSTM directions packed into shared [40, *] tiles (left chain on
partitions 0:8, right chain on 32:40) so each activation / DVE / Pool
instruction covers both chains at once.

Gate columns use a slice-interleaved layout: PSUM chunk s (widths 512/512/176,
one PSUM tile each so sigmoids only depend on their own chunk's matmuls)
holds gates (g,i,f,o) for hidden-slice s (widths 128/128/44). g-gate columns
are pre-scaled by 2 so sigmoid(2g) stands in for tanh via 2s-1 (one fused
tensor_scalar). The cell update is c = f*c + i*(2s-1); h = o*tanh(c).

The recurrent-matmul operand hT is built without h on the critical path:
o-gates are transposed right after their sigmoid (tpA, off-path), tanh(c)
slices are transposed as they appear (tpB), and hT = oT * tcT on DVE. The
PE stream per step is [k3-group, k1-group, k2-group, transposes]; the next
step's matmuls bypass not-yet-ready transposes via the PE wait queue, so
each slice's elementwise tail overlaps the following step's matmul phase.
x-inputs are staged 8 steps per DMA into wide SBUF tiles (whh3 rows
replicated per step-block); h outputs accumulate in SBUF and store 8 steps
per DMA. GPSIMD (Pool) runs only SBUF-to-SBUF elementwise (it cannot access
PSUM); PSUM reads go to DVE/Act.
"""

import os
import sys

import numpy as np

sys.path.insert(0, "/opt/trn_rl_repo")

import concourse.bass as bass
import concourse.bacc as bacc
import concourse.mybir as mybir
import concourse.tile as tile
from concourse.bass_utils import run_bass_kernel_spmd
from concourse.masks import make_identity

F32 = mybir.dt.float32
BF16 = mybir.dt.bfloat16
I32 = mybir.dt.int32
AF = mybir.ActivationFunctionType
ALU = mybir.AluOpType

B_TOT, S = 64, 512
DC = DB = 200
HID = H = 300
VC, VB = 10000, 200000
NCORES = 8
BL = B_TOT // NCORES          # 8 sentences per core
T = BL * S                    # 4096 tokens per core
G4 = 4 * H                    # 1200
N_TILES = T // 128            # 32 prep tiles
STEPS = S

M300 = [128, 128, 44]         # chunks of 300 (lin output dims)
KXP = [128, 128, 65]          # xproj contraction chunks (65 = 44 dims + ones@64)
RB = 32                       # right chain's partition base (32-aligned)

# recurrence layout
W3 = [128, 128, 44]           # hidden-slice widths
HOFF = [0, 128, 256]          # hidden-slice offsets
Q3 = [0, 512, 1024]           # gate-chunk column bases (chunk s = slice s)
KBLK = 8                      # steps per x-staging / h-store block
NBLK = S // KBLK
# slice processing order: slice 2 first (it feeds the k3 pass of the next
# step's matmuls, which leads the PE stream), then 0 (k1), then 1 (k2)
SORDER = [2, 0, 1]
GG, GI, GF, GO = 0, 1, 2, 3   # gate sub-order within a chunk


def _gcol(s, gate):
    return Q3[s] + gate * W3[s]


def _build_program():
    nc = bacc.Bacc()

    idx_d = nc.declare_dram_parameter("idx", [128, N_TILES * 8], I32, isOutput=False)
    tab_char = nc.declare_dram_parameter("char_embed", [VC, DC], F32, isOutput=False)
    tab_schar = nc.declare_dram_parameter("static_char_embed", [VC, DC], F32, isOutput=False)
    tab_bi = nc.declare_dram_parameter("bichar_embed", [VB, DB], F32, isOutput=False)
    tab_sbi = nc.declare_dram_parameter("static_bichar_embed", [VB, DB], F32, isOutput=False)
    wlin_d = nc.declare_dram_parameter("wlin_blk", [128, 24 * 128], BF16, isOutput=False)
    blin_d = nc.declare_dram_parameter("blin_blk", [128, 3], F32, isOutput=False)
    wih_d = nc.declare_dram_parameter("wihaug_blk", [128, 2 * 3 * G4], BF16, isOutput=False)
    whh12_d = nc.declare_dram_parameter("whh12_blk", [128, 2 * 2 * G4], BF16, isOutput=False)
    whh3rep_d = nc.declare_dram_parameter("whh3rep_blk", [44, 2 * KBLK * G4], BF16, isOutput=False)
    i8w_d = nc.declare_dram_parameter("i8wblk", [8, 40], BF16, isOutput=False)
    ones_d = nc.declare_dram_parameter("onesblk", [1, 128], BF16, isOutput=False)
    hs_d = nc.declare_dram_parameter("hs", [2, T, H], F32, isOutput=True)
    x_d = nc.dram_tensor("x_seq", [2, T, G4], BF16)

    tables = [tab_char, tab_schar, tab_bi, tab_sbi]

    with tile.TileContext(nc) as tc:
        with (
            tc.tile_pool(name="const", bufs=1) as cp,
            tc.tile_pool(name="ph_sb", bufs=2) as pp,
            tc.tile_pool(name="rc_sb", bufs=2) as rp,
            tc.tile_pool(name="rc_h", bufs=2) as hp,
            tc.tile_pool(name="ps", bufs=1, space="PSUM") as psp,
        ):
            ident = cp.tile([128, 128], F32, tag="ident")
            make_identity(nc, ident[:, :])
            idx_sb = cp.tile([128, N_TILES * 8], I32, tag="idx")
            nc.sync.dma_start(out=idx_sb[:, :], in_=idx_d[:, :])
            wlin_sb = cp.tile([128, 24 * 128], BF16, tag="wlin")
            nc.sync.dma_start(out=wlin_sb[:, :], in_=wlin_d[:, :])
            blin_sb = cp.tile([128, 3], F32, tag="blin")
            nc.sync.dma_start(out=blin_sb[:, :], in_=blin_d[:, :])
            wih_sb = cp.tile([128, 2 * 3 * G4], BF16, tag="wih")
            nc.sync.dma_start(out=wih_sb[:, :], in_=wih_d[:, :])
            whh12_sb = cp.tile([128, 2 * 2 * G4], BF16, tag="whh12")
            nc.sync.dma_start(out=whh12_sb[:, :], in_=whh12_d[:, :])

            # persistent linT tiles (side x parity); ones row 64 loaded once
            linTs = {}
            for side in range(2):
                for par in range(2):
                    lt = cp.tile([128, 3 * 128], BF16, tag=f"linT_{side}_{par}")
                    nc.sync.dma_start(out=lt[64:65, 256:384], in_=ones_d[:, :])
                    linTs[(side, par)] = lt

            # recurrence state: packed both chains (L rows 0:8, R rows 32:40)
            hT1 = cp.tile([128, 40], BF16, tag="hT1")
            nc.vector.memset(hT1[:, :], 0.0)
            hT2 = cp.tile([128, 40], BF16, tag="hT2")
            nc.vector.memset(hT2[:, :], 0.0)
            hT3e = cp.tile([52, 40], BF16, tag="hT3e")
            nc.vector.memset(hT3e[0:44, :], 0.0)
            nc.sync.dma_start(out=hT3e[44:52, :], in_=i8w_d[:, :])
            c_st = cp.tile([40, H], F32, tag="c_st")
            nc.vector.memset(c_st[:, :], 0.0)

            # wide x staging: rows 0:44 whh3 (replicated per step), 44:52 x(t)
            b3w = []
            for c in range(2):
                bufs = []
                for r in range(2):
                    bw = cp.tile([52, KBLK * G4], BF16, tag=f"b3w_{c}_{r}")
                    nc.sync.dma_start(
                        out=bw[0:44, :],
                        in_=whh3rep_d[:, c * KBLK * G4:(c + 1) * KBLK * G4])
                    bufs.append(bw)
                b3w.append(bufs)

            # ---------------- phases 1-3: gather, transpose, linear, xproj ----
            for t in range(N_TILES):
                for side in range(2):
                    feat = pp.tile([128, 800], F32, tag=f"feat{side}")
                    for j4 in range(4):
                        col = t * 8 + side * 4 + j4
                        nc.gpsimd.indirect_dma_start(
                            out=feat[:, 200 * j4:200 * (j4 + 1)],
                            out_offset=None,
                            in_=tables[j4][:, :],
                            in_offset=bass.IndirectOffsetOnAxis(
                                ap=idx_sb[:, col:col + 1], axis=0),
                        )
                    # 8 transposes of 100-col slices (each inside one gather segment)
                    featT = pp.tile([128, 8 * 128], BF16, tag=f"ft{side}")
                    for kc in range(8):
                        tp = psp.tile([128, 128], F32, tag="plg1", bufs=2)
                        nc.tensor.transpose(
                            tp[0:100, 0:128], feat[:, kc * 100:(kc + 1) * 100],
                            ident[:, :])
                        nc.vector.tensor_copy(
                            featT[0:100, kc * 128:(kc + 1) * 128], tp[0:100, 0:128])
                    linT = linTs[(side, t % 2)]
                    for m in range(3):
                        mm = M300[m]
                        pl = psp.tile([128, 128], F32, tag="plg1", bufs=2)
                        for kc in range(8):
                            blk = (kc * 3 + m) * 128
                            nc.tensor.matmul(
                                pl[0:mm, 0:128],
                                lhsT=wlin_sb[0:100, blk:blk + mm],
                                rhs=featT[0:100, kc * 128:(kc + 1) * 128],
                                start=(kc == 0), stop=(kc == 7))
                        nc.scalar.activation(
                            linT[0:mm, m * 128:m * 128 + 128],
                            pl[0:mm, 0:128], AF.Tanh,
                            bias=blin_sb[0:mm, m:m + 1])
                    pxc0 = psp.tile([128, 512], F32, tag="pxg0", bufs=2)
                    pxc1 = psp.tile([128, 512], F32, tag="pxg1", bufs=2)
                    pxc2 = psp.tile([128, 176], F32, tag="pxg2", bufs=2)
                    pxc = [pxc0, pxc1, pxc2]
                    for kc in range(3):
                        kw = KXP[kc]
                        for s, (n0, nw) in enumerate(
                                [(0, 512), (512, 512), (1024, 176)]):
                            nc.tensor.matmul(
                                pxc[s][:, 0:nw],
                                lhsT=linT[0:kw, kc * 128:kc * 128 + 128],
                                rhs=wih_sb[0:kw, (side * 3 + kc) * G4 + n0:
                                           (side * 3 + kc) * G4 + n0 + nw],
                                start=(kc == 0), stop=(kc == 2))
                    x_sb = pp.tile([128, G4], BF16, tag=f"x{side}")
                    for s, (n0, nw) in enumerate(
                            [(0, 512), (512, 512), (1024, 176)]):
                        nc.scalar.copy(x_sb[:, n0:n0 + nw], pxc[s][:, 0:nw])
                    nc.sync.dma_start(
                        out=x_d[side, t * 128:(t + 1) * 128, :], in_=x_sb[:, :])

            # ---------------- phase 4: packed bi-LSTM recurrence --------------
            # prologue: stage x block 0
            for c in range(2):
                nc.sync.dma_start(
                    out=b3w[c][0][44:52, :].rearrange("b (k d) -> b k d", k=KBLK),
                    in_=x_d[c, 0:KBLK * 8, :].rearrange("(k b) d -> b k d", b=8))

            hTs = [hT1, hT2, hT3e]
            # chunk emission order within each k-group: chunk2 first
            CORDER = [2, 0, 1]
            hacc_cur = None

            # oT staging tiles (o-gate transposed early, off the critical path)
            oTs = []
            for s in range(3):
                oT_t = cp.tile([128, 40], F32, tag=f"oT{s}")
                oTs.append(oT_t)

            for t in range(STEPS):
                blk, j = divmod(t, KBLK)
                if j == 0:
                    hacc_cur = hp.tile([40, KBLK * H], F32, tag="hacc")
                    if blk + 1 < NBLK:
                        nb = (blk + 1) % 2
                        for c in range(2):
                            nc.sync.dma_start(
                                out=b3w[c][nb][44:52, :].rearrange(
                                    "b (k d) -> b k d", k=KBLK),
                                in_=x_d[c, (blk + 1) * KBLK * 8:
                                        (blk + 2) * KBLK * 8, :].rearrange(
                                    "(k b) d -> b k d", b=8))

                # separate PSUM tile per gate chunk so sigmoid s only
                # depends on chunk s's matmuls (tile-granular dep tracking)
                Gc0 = psp.tile([128, 512], F32, tag="pxg0", bufs=2)
                Gc1 = psp.tile([128, 512], F32, tag="pxg1", bufs=2)
                Gc2 = psp.tile([128, 176], F32, tag="pxg2", bufs=2)
                Gc = [Gc0, Gc1, Gc2]

                # k3 group (whh rows 256:300 + x via identity rows), start=True
                buf = blk % 2
                for ci in CORDER:
                    n0, nw = Q3[ci], 4 * W3[ci]
                    for c in range(2):
                        cb = c * RB
                        nc.tensor.matmul(
                            Gc[ci][cb:cb + 8, 0:nw],
                            lhsT=hT3e[0:52, cb:cb + 8],
                            rhs=b3w[c][buf][0:52, j * G4 + n0:j * G4 + n0 + nw],
                            start=True, stop=False, skip_group_check=True)
                # k1 group (whh rows 0:128)
                for ci in CORDER:
                    n0, nw = Q3[ci], 4 * W3[ci]
                    for c in range(2):
                        cb = c * RB
                        nc.tensor.matmul(
                            Gc[ci][cb:cb + 8, 0:nw],
                            lhsT=hT1[:, cb:cb + 8],
                            rhs=whh12_sb[:, (c * 2) * G4 + n0:(c * 2) * G4 + n0 + nw],
                            start=False, stop=False, skip_group_check=True)
                # k2 group (whh rows 128:256), stop=True
                for ci in CORDER:
                    n0, nw = Q3[ci], 4 * W3[ci]
                    for c in range(2):
                        cb = c * RB
                        nc.tensor.matmul(
                            Gc[ci][cb:cb + 8, 0:nw],
                            lhsT=hT2[:, cb:cb + 8],
                            rhs=whh12_sb[:, (c * 2 + 1) * G4 + n0:
                                         (c * 2 + 1) * G4 + n0 + nw],
                            start=False, stop=True, skip_group_check=True)

                # activations + elementwise tails; slice 2 first (it gates the
                # next step's k3 group), then 0 (k1), then 1 (k2).
                sg = rp.tile([40, G4], F32, tag="sg")
                gt = rp.tile([40, H], F32, tag="gt")
                t1 = rp.tile([40, H], F32, tag="t1")
                t2 = rp.tile([40, H], F32, tag="t2")
                tcc = rp.tile([40, H], F32, tag="tc")

                def sig(s):
                    # one sigmoid per chunk: each extra Act instruction costs
                    # ~220ns fixed overhead, and Act is the capacity-limiting
                    # engine in the recurrence
                    nw = 4 * W3[s]
                    nc.scalar.activation(sg[:, Q3[s]:Q3[s] + nw],
                                         Gc[s][0:40, 0:nw], AF.Sigmoid)

                def tail_pre(s):
                    w, ho = W3[s], HOFF[s]
                    hsl = slice(ho, ho + w)
                    # slice 2 is the T-critical tail: run it back-to-back on
                    # DVE (no cross-engine hops); slices 0/1 split Pool/DVE
                    eng_a = nc.vector if s == 2 else nc.gpsimd
                    # g~ = 2*sigmoid(2g) - 1 = tanh(g)   [fused]
                    eng_a.tensor_scalar(
                        gt[:, hsl], sg[:, _gcol(s, GG):_gcol(s, GG) + w],
                        2.0, -1.0, ALU.mult, ALU.add)
                    # t1 = f * c
                    eng_a.tensor_tensor(
                        t1[:, hsl], sg[:, _gcol(s, GF):_gcol(s, GF) + w],
                        c_st[:, hsl], op=ALU.mult)
                    # t2 = i * g~                         [DVE]
                    nc.vector.tensor_tensor(
                        t2[:, hsl], sg[:, _gcol(s, GI):_gcol(s, GI) + w],
                        gt[:, hsl], op=ALU.mult)
                    # c = t1 + t2                         [DVE]
                    nc.vector.tensor_tensor(
                        c_st[:, hsl], t1[:, hsl], t2[:, hsl], op=ALU.add)

                def tail_post(s):
                    w, ho = W3[s], HOFF[s]
                    hsl = slice(ho, ho + w)
                    # tc = tanh(c)                        [Act]
                    nc.scalar.activation(tcc[:, hsl], c_st[:, hsl], AF.Tanh)
                    # h = o * tc -> hacc (output store only, off the critical
                    # path: hT for the next matmul comes from tpB/multT) [DVE]
                    nc.vector.tensor_tensor(
                        hacc_cur[:, j * H + ho:j * H + ho + w],
                        sg[:, _gcol(s, GO):_gcol(s, GO) + w],
                        tcc[:, hsl], op=ALU.mult)

                def tp_a(s):
                    # transpose the o-gate right after its sigmoid [PE+DVE]
                    # (GPSIMD cannot touch PSUM on real hardware)
                    w = W3[s]
                    tpa = psp.tile([128, 128], F32, tag="plg1", bufs=2)
                    nc.tensor.transpose(
                        tpa[0:w, 0:40],
                        sg[:, _gcol(s, GO):_gcol(s, GO) + w], ident[0:40, 0:40])
                    nc.vector.tensor_copy(oTs[s][0:w, 0:40], tpa[0:w, 0:40])

                def tp_b(s):
                    # transpose tanh(c), then hT = oT * tcT  [PE + DVE]
                    w, ho = W3[s], HOFF[s]
                    tpb = psp.tile([128, 128], F32, tag="plg1", bufs=2)
                    nc.tensor.transpose(
                        tpb[0:w, 0:40], tcc[:, ho:ho + w], ident[0:40, 0:40])
                    dst = hT3e[0:44, 0:40] if s == 2 else hTs[s][0:w, 0:40]
                    nc.vector.tensor_tensor(
                        dst, oTs[s][0:w, 0:40], tpb[0:w, 0:40], op=ALU.mult)

                last = t == STEPS - 1
                sig(2)
                if not last:
                    tp_a(2)
                tail_pre(2)
                sig(0)
                if not last:
                    tp_a(0)
                tail_post(2)
                if not last:
                    tp_b(2)
                tail_pre(0)
                sig(1)
                if not last:
                    tp_a(1)
                tail_post(0)
                if not last:
                    tp_b(0)
                tail_pre(1)
                tail_post(1)
                if not last:
                    tp_b(1)

                if j == KBLK - 1:
                    for c in range(2):
                        cb = c * RB
                        nc.sync.dma_start(
                            out=hs_d[c, blk * KBLK * 8:(blk + 1) * KBLK * 8,
                                     :].rearrange("(k b) d -> b k d", b=8),
                            in_=hacc_cur[cb:cb + 8, :].rearrange(
                                "b (k d) -> b k d", k=KBLK))
    nc.compile()
    return nc


def _prep_host(inputs):
    """Build the per-core in_maps (host-side weight/index preprocessing)."""
    f = {k: np.asarray(v) for k, v in inputs.items()}

    wlinT = f["W_lin"].astype(np.float32).T            # [800, 300]
    wlin_blk = np.zeros((128, 24 * 128), np.float32)
    for kc in range(8):
        for m in range(3):
            mm = M300[m]
            blk = (kc * 3 + m) * 128
            wlin_blk[0:100, blk:blk + mm] = wlinT[kc * 100:(kc + 1) * 100,
                                                 m * 128:m * 128 + mm]
    blin_blk = np.zeros((128, 3), np.float32)
    for m in range(3):
        mm = M300[m]
        blin_blk[0:mm, m] = f["b_lin"][m * 128:m * 128 + mm]

    # gate-column permutation: chunk s holds (g,i,f,o) for hidden slice s;
    # original torch gate order is (i,f,g,o)
    P = np.zeros(G4, np.int64)
    gscale = np.ones(G4, np.float32)
    orig_base = {GG: 600, GI: 0, GF: 300, GO: 900}
    for s in range(3):
        for gate in range(4):
            cols = np.arange(W3[s])
            P[Q3[s] + gate * W3[s] + cols] = orig_base[gate] + HOFF[s] + cols
            if gate == GG:
                gscale[Q3[s] + gate * W3[s] + cols] = 2.0

    wih_blk = np.zeros((128, 2 * 3 * G4), np.float32)
    whh12_blk = np.zeros((128, 2 * 2 * G4), np.float32)
    whh3rep_blk = np.zeros((44, 2 * KBLK * G4), np.float32)
    for c, sfx in enumerate(("l", "r")):
        wihT = (f[f"Wih_{sfx}"][P, :].astype(np.float32) * gscale[:, None]).T
        bb = f[f"b_{sfx}"][P].astype(np.float32) * gscale
        for kc in range(2):
            wih_blk[0:128, (c * 3 + kc) * G4:(c * 3 + kc + 1) * G4] = \
                wihT[kc * 128:(kc + 1) * 128, :]
        wih_blk[0:44, (c * 3 + 2) * G4:(c * 3 + 3) * G4] = wihT[256:300, :]
        wih_blk[64, (c * 3 + 2) * G4:(c * 3 + 3) * G4] = bb
        whhT = (f[f"Whh_{sfx}"][P, :].astype(np.float32) * gscale[:, None]).T
        whh12_blk[:, (c * 2) * G4:(c * 2 + 1) * G4] = whhT[0:128, :]
        whh12_blk[:, (c * 2 + 1) * G4:(c * 2 + 2) * G4] = whhT[128:256, :]
        whh3rep_blk[:, c * KBLK * G4:(c + 1) * KBLK * G4] = \
            np.tile(whhT[256:300, :], (1, KBLK))

    i8w = np.zeros((8, 40), np.float32)
    i8w[:, 0:8] = np.eye(8)
    i8w[:, 32:40] = np.eye(8)

    import ml_dtypes
    bf = lambda a: a.astype(ml_dtypes.bfloat16)
    shared = {
        "char_embed": f["char_embed"].astype(np.float32),
        "static_char_embed": f["static_char_embed"].astype(np.float32),
        "bichar_embed": f["bichar_embed"].astype(np.float32),
        "static_bichar_embed": f["static_bichar_embed"].astype(np.float32),
        "wlin_blk": bf(wlin_blk), "blin_blk": blin_blk,
        "wihaug_blk": bf(wih_blk), "whh12_blk": bf(whh12_blk),
        "whh3rep_blk": bf(whh3rep_blk),
        "i8wblk": bf(i8w),
        "onesblk": bf(np.ones((1, 128), np.float32)),
    }

    in_maps = []
    for core in range(NCORES):
        bs = slice(core * BL, (core + 1) * BL)
        idx_blk = np.zeros((128, N_TILES * 8), np.int32)
        # stream order: [charL scharL bilL sbilL | charR scharR birR sbirR]
        streams = [
            f["char_features"][bs].T.reshape(-1),
            f["static_char_features"][bs].T.reshape(-1),
            f["bichar_left_features"][bs].T.reshape(-1),
            f["static_bichar_left_features"][bs].T.reshape(-1),
            f["char_features"][bs][:, ::-1].T.reshape(-1),
            f["static_char_features"][bs][:, ::-1].T.reshape(-1),
            f["bichar_right_features"][bs][:, ::-1].T.reshape(-1),
            f["static_bichar_right_features"][bs][:, ::-1].T.reshape(-1),
        ]
        for t in range(N_TILES):
            for jj in range(8):
                idx_blk[:, t * 8 + jj] = streams[jj][t * 128:(t + 1) * 128]
        in_maps.append({"idx": idx_blk, **shared})
    return in_maps


_CACHED = {}


def kernel(**inputs):
    if "nc" not in _CACHED:
        _CACHED["nc"] = _build_program()
    nc = _CACHED["nc"]
    in_maps = _prep_host(inputs)
    trace = bool(os.environ.get("K_TRACE"))
    res = run_bass_kernel_spmd(nc, in_maps, list(range(NCORES)), trace=trace)
    if trace:
        _CACHED["exec_time_ns"] = res.exec_time_ns
        _CACHED["trace_path"] = (res.instructions_and_trace or (None, None))[1]
    out = np.empty((B_TOT, S, 2 * H), np.float32)
    for core in range(NCORES):
        hs = res.results[core]["hs"].reshape(2, S, BL, H)
        bs = slice(core * BL, (core + 1) * BL)
        out[bs, :, 0:H] = hs[0].transpose(1, 0, 2)
        out[bs, :, H:2 * H] = hs[1, ::-1].transpose(1, 0, 2)
    return out


if __name__ == "__main__":
    sys.path.insert(0, os.path.dirname(os.path.abspath(__file__)))
    import reference
    inp = reference.setup_inputs()
    got = kernel(**{k: np.asarray(v) for k, v in inp.items()})
    exp = np.asarray(reference.reference(**inp))
    err = np.abs(got - exp)
    rel = err.max() / np.abs(exp).max()
    print("Relative error:", rel)
